# revision 20
# baseline (speedup 1.0000x reference)
"""Trainium2 Bass kernel for nn_FBSDEModel_MFG (mean-field FBSDE simulation).

Math (reference semantics, CA=1):
    y0 = MLP_y(x)                        # 1->64->64->1, relu
    per step t (dt = 1/T):
      m_t    = mean(y_t)  over ALL samples          <-- only cross-core term
      y_diff = MLP_z([t*dt, x_t])                   # 2->128->128->1, relu
      x_{t+1} = x_t - y_t*dt + SIGMA*dw_t
      y_{t+1} = y_t - (CX*x_t + GAMMA*m_t)*dt + y_diff*dw_t
    out: (y_T, CG*x_T)

Decomposition used on device (exact algebra): y = u + c, x = xl + d with
c,d global scalars identical on every core:
      u_{t+1}  = u_t - CX*dt*xl_t + y_diff*dw_t          (local)
      xl_{t+1} = xl_t - dt*u_t + SIGMA*dw_t              (local)
      c_{t+1}  = (1-GAMMA*dt)c_t - CX*dt*d_t - (GAMMA*dt/N)*Sg_t
      d_{t+1}  = d_t - dt*c_t
      Sg_t     = sum over ALL samples of u_t   <-- AllGather of per-core sums
      MLP_z input x_t = xl_t + d_t  (d folded in as a shift before layer 1)

Sharding: pure data-parallel over 8 cores, 2048 samples each. The per-step
scalar AllGather (4B/core) is the only collective; it has ~1 full step of
compute to hide under.

Layouts: z-net activations are feature-major [feat, sample]; per-sample
state lives as [128, n_chunk] columns (sample s = chunk*128 + partition),
which is exactly the layout the chunked output-layer matmuls produce.
"""

import os
import sys

sys.path.insert(0, "/opt/trn_rl_repo")

import numpy as np

# model constants (must match reference.py)
CA = 1.0
CX = 0.1
GAMMA = 0.5
SIGMA = 0.7
CG = 1.0
MATURITY = 1.0

N_CORES = 8

_BUILD_CACHE = {}
_LAST_IN_MAPS = None


def _build(T, n_loc, b3y, b3z, NTOT):
    """Build + compile the SPMD bass program. Parametric in T and n_loc."""
    from concourse import bacc, mybir
    import concourse.tile as tile

    dt32 = mybir.dt.float32
    dt32r = mybir.dt.float32r
    AO = mybir.AluOpType
    Relu = mybir.ActivationFunctionType.Relu

    dtv = float(MATURITY) / T
    nch = n_loc // 128          # state columns (16 for n_loc=2048)
    n_free = n_loc              # z-net free dim per step
    nt512 = n_free // 512       # 512-wide matmul tiles (4)

    _rep = int(os.environ.get("REPDEV", "0")) > 1
    GR = int(os.environ.get("PSUM_GR", "1024"))
    _xb = 2 if not _rep else 4
    _sb = 3 if not _rep else 5
    nc = bacc.Bacc("TRN2", target_bir_lowering=False, debug=False,
                   num_devices=N_CORES)

    # ---- DRAM I/O ----
    D = {}
    def din(name, shape, dt=dt32):
        D[name] = nc.dram_tensor(name, shape, dt, kind="ExternalInput")
        return D[name]

    din("xrow_in", [1, n_loc], dt32r)
    din("xcol_in", [128, nch])
    din("dwcol", [128, T * nch])
    din("sdwcol", [128, T * nch])
    din("w1x", [1, 128], dt32r)
    din("w1xf", [1, 128])
    din("b1t", [128, T])
    din("W2z", [128, 128], dt32r)
    din("w3z", [128, 2], dt32r)
    din("b2z", [128, 1])
    din("w1y", [1, 64], dt32r)
    din("b1y", [64, 1])
    din("W2y", [64, 64], dt32r)
    din("b2y", [64, 1])
    din("w3y", [64, 2], dt32r)
    din("Amat", [10, 2])
    din("onesc", [128, 1])
    din("onesr", [1, 128])
    din("ident", [128, 128])
    yout_d = nc.dram_tensor("yout", [128, nch], dt32, kind="ExternalOutput")
    xout_d = nc.dram_tensor("xout", [128, nch], dt32, kind="ExternalOutput")

    r = lambda ap: ap

    with tile.TileContext(nc) as tc:
        with (
            tc.tile_pool(name="const", bufs=1) as cpool,
            tc.tile_pool(name="state", bufs=_sb) as spool,
            tc.tile_pool(name="hpool", bufs=2 if not _rep else 4) as hpool,
            tc.tile_pool(name="xrowp", bufs=_xb) as xrowp,
            tc.tile_pool(name="tmp", bufs=_sb) as tmpp,
            tc.tile_pool(name="psmm", bufs=(3 if GR == 1024 else 6), space="PSUM") as psmm,
            tc.tile_pool(name="pssm", bufs=2, space="PSUM") as pssm,
            tc.tile_pool(name="dram", bufs=4, space="DRAM") as dpool,
        ):
            # ---- load constants ----
            C = {}
            for name, shape in [
                ("w1x", [1, 128]), ("w1xf", [1, 128]), ("b1t", [128, T]), ("W2z", [128, 128]),
                ("w3z", [128, 2]), ("b2z", [128, 1]), ("w1y", [1, 64]),
                ("b1y", [64, 1]), ("W2y", [64, 64]), ("b2y", [64, 1]),
                ("w3y", [64, 2]), ("Amat", [10, 2]), ("onesc", [128, 1]),
                ("onesr", [1, 128]), ("ident", [128, 128]),
                ("dwcol", [128, T * nch]), ("sdwcol", [128, T * nch]),
            ]:
                cdt = dt32r if name in ("w1x", "W2z", "w3z", "w1y", "W2y", "w3y") else dt32
                C[name] = cpool.tile(shape, cdt, tag=f"c_{name}", name=f"c_{name}")
                nc.sync.dma_start(C[name][:], D[name][:])

            # scalar state (c, d) as a [1,2] row on partition 0
            scd = tmpp.tile([1, 2], dt32, tag="scd")
            nc.vector.memset(scd[:], 0.0)

            # ---- initial x row + column state ----
            xrow0 = xrowp.tile([1, n_loc], dt32r, tag="xrow")
            nc.sync.dma_start(xrow0[:], D["xrow_in"][:])
            xt = spool.tile([128, nch], dt32, tag="xt")
            nc.sync.dma_start(xt[:], D["xcol_in"][:])

            # ---- y-net: u0 = MLP_y(x) ----
            h1y = tmpp.tile([64, n_free], dt32r, tag="h1y")
            h2y = tmpp.tile([64, n_free], dt32r, tag="h2y")
            for h in range(n_free // GR):
                py = psmm.tile([64, GR], dt32, tag="mm")
                for j in range(GR // 512):
                    s = h * GR + j * 512
                    nc.tensor.matmul(py[:, j * 512:(j + 1) * 512],
                                     r(C["w1y"][:]), r(xrow0[0:1, s:s + 512]),
                                     start=True, stop=True)
                nc.scalar.activation(h1y[:, h * GR:(h + 1) * GR], py[:],
                                     Relu, bias=C["b1y"][:, 0:1])
            for h in range(n_free // GR):
                py = psmm.tile([64, GR], dt32, tag="mm")
                for j in range(GR // 512):
                    s = h * GR + j * 512
                    nc.tensor.matmul(py[:, j * 512:(j + 1) * 512],
                                     r(C["W2y"][:]), r(h1y[:, s:s + 512]),
                                     start=True, stop=True)
                nc.scalar.activation(h2y[:, h * GR:(h + 1) * GR], py[:],
                                     Relu, bias=C["b2y"][:, 0:1])
            py0 = pssm.tile([128, 2 * nch], dt32, tag="sm")
            for k in range(nch):
                nc.tensor.matmul(py0[:, 2 * k:2 * k + 2],
                                 r(h2y[:, k * 128:(k + 1) * 128]),
                                 r(C["w3y"][:]), start=True, stop=True)
            ut = spool.tile([128, nch], dt32, tag="ut")
            uacc = tmpp.tile([128, 1], dt32, tag="uacc")
            nc.vector.tensor_scalar(ut[:], py0[:, 0:2 * nch:2], float(b3y),
                                    0.0, AO.add, AO.add, accum_out=uacc[:])

            NOCOLL = os.environ.get("NOCOLL", "0") == "1"
            # ---- S_0 partial sum + AllGather launch ----
            def launch_ag(uacc_tile):
                ps = pssm.tile([1, 1], dt32, tag="sm")
                nc.tensor.matmul(ps[:], r(C["onesc"][:]), r(uacc_tile[:]),
                                 start=True, stop=True)
                ssb = tmpp.tile([1, 8], dt32, tag="ssb")
                nc.vector.tensor_copy(ssb[0:1, 0:1], ps[:])
                agin = dpool.tile([1, 8], dt32, tag="agin")
                agout = dpool.tile([8, 8], dt32, tag="agout")
                nc.gpsimd.dma_start(agin[0:1, 0:1], ssb[0:1, 0:1])
                if NOCOLL:
                    nc.sync.dma_start(agout[0:1, :], agin[0:1, :])
                else:
                    nc.gpsimd.collective_compute(
                        "AllGather", AO.bypass,
                        replica_groups=[list(range(N_CORES))],
                        ins=[agin.opt()], outs=[agout.opt()],
                    )
                return agout

            agout_t = launch_ag(uacc)

            REPDEV = int(os.environ.get("REPDEV", "0"))
            rep_ctx = tc.For_i(0, REPDEV, 1) if REPDEV > 1 else None
            if rep_ctx is not None:
                rep_ctx.__enter__()

            xrow_t = xrow0
            ceff_t = None           # step-0 bias handled via b1t directly
            for t in range(T):
                dwt = C["dwcol"][:, t * nch:(t + 1) * nch]
                sdwt = C["sdwcol"][:, t * nch:(t + 1) * nch]

                # ---- x-state advance (needs only step t-1 data) ----
                t3 = tmpp.tile([128, nch], dt32, tag="t3")
                nc.vector.scalar_tensor_tensor(t3[:], ut[:], -dtv, sdwt,
                                               AO.mult, AO.add)
                xt_new = spool.tile([128, nch], dt32, tag="xt")
                nc.vector.tensor_add(xt_new[:], xt[:], t3[:])

                # ---- bridge for step t+1 (off critical path) ----
                if t < T - 1:
                    pT = pssm.tile([nch, 128], dt32, tag="sm")
                    nc.tensor.transpose(pT[:], xt_new[:], C["ident"][:])
                    sT = tmpp.tile([nch, 128], dt32r, tag="sT")
                    nc.vector.tensor_copy(sT[:], pT[:])
                    xrow_next = xrowp.tile([1, n_loc], dt32r, tag="xrow")
                    nc.sync.dma_start(xrow_next[0:1, :], sT[:, :])

                # ---- z-net layer 1 (rank-1) + relu (bias carries d_t) ----
                bias1 = C["b1t"][:, t:t + 1] if ceff_t is None else ceff_t[:, 0:1]
                h1 = hpool.tile([128, n_free], dt32r, tag="h1")
                for h in range(n_free // GR):
                    p1 = psmm.tile([128, GR], dt32, tag="mm")
                    for j in range(GR // 512):
                        s = h * GR + j * 512
                        nc.tensor.matmul(p1[:, j * 512:(j + 1) * 512],
                                         r(C["w1x"][:]),
                                         r(xrow_t[0:1, s:s + 512]),
                                         start=True, stop=True)
                    nc.scalar.activation(h1[:, h * GR:(h + 1) * GR],
                                         p1[:], Relu, bias=bias1)

                # ---- z-net layer 2 + relu (split ACT/DVE) ----
                h2 = hpool.tile([128, n_free], dt32r, tag="h2")
                for h in range(n_free // GR):
                    p2 = psmm.tile([128, GR], dt32, tag="mm")
                    for j in range(GR // 512):
                        s = h * GR + j * 512
                        nc.tensor.matmul(p2[:, j * 512:(j + 1) * 512],
                                         r(C["W2z"][:]), r(h1[:, s:s + 512]),
                                         start=True, stop=True)
                    if h >= (n_free // GR) // 2:
                        nc.scalar.activation(h2[:, h * GR:(h + 1) * GR],
                                             p2[:], Relu, bias=C["b2z"][:, 0:1])
                    else:
                        nc.vector.tensor_scalar(h2[:, h * GR:(h + 1) * GR],
                                                p2[:], C["b2z"][:, 0:1], 0.0,
                                                AO.add, AO.max)

                # ---- z-net layer 3, chunked into state layout ----
                pyd = pssm.tile([128, 2 * nch], dt32, tag="sm")
                for k in range(nch):
                    nc.tensor.matmul(pyd[:, 2 * k:2 * k + 2],
                                     r(h2[:, k * 128:(k + 1) * 128]),
                                     r(C["w3z"][:]), start=True, stop=True)

                # ---- u-state update ----
                q = tmpp.tile([128, nch], dt32, tag="q")
                nc.vector.scalar_tensor_tensor(q[:], pyd[:, 0:2 * nch:2],
                                               float(b3z), dwt, AO.add,
                                               AO.mult)
                t2 = tmpp.tile([128, nch], dt32, tag="t2")
                nc.vector.scalar_tensor_tensor(t2[:], xt[:], -CX * dtv, ut[:],
                                               AO.mult, AO.add)
                ut_new = spool.tile([128, nch], dt32, tag="ut")
                uacc = tmpp.tile([128, 1], dt32, tag="uacc")
                nc.vector.scalar_tensor_tensor(ut_new[:], t2[:], 1.0, q[:],
                                               AO.mult, AO.add,
                                               accum_out=uacc[:])
                ut, xt = ut_new, xt_new
                if t < T - 1:
                    xrow_t = xrow_next

                # ---- launch AllGather of S_{t+1} ----
                if t < T - 1:
                    agout_next = launch_ag(uacc)

                # ---- consume AllGather_t: scalar recursion (partition-0 row) ----
                sgrow = tmpp.tile([1, 8], dt32, tag="sgrow")
                nc.gpsimd.dma_start(sgrow[0:1, :], agout_t[0:8, 0:1])
                sg1 = tmpp.tile([1, 1], dt32, tag="sg1")
                nc.vector.tensor_reduce(sg1[:], sgrow[:], mybir.AxisListType.X,
                                        AO.add)
                scd_new = tmpp.tile([1, 2], dt32, tag="scd")
                ta = tmpp.tile([1, 1], dt32, tag="ta")
                nc.vector.tensor_scalar(ta[:], scd[0:1, 0:1],
                                        1.0 - GAMMA * dtv, None, AO.mult)
                tb = tmpp.tile([1, 1], dt32, tag="tb")
                nc.vector.scalar_tensor_tensor(tb[:], scd[0:1, 1:2], -CX * dtv,
                                               ta[:], AO.mult, AO.add)
                nc.vector.scalar_tensor_tensor(scd_new[0:1, 0:1], sg1[:],
                                               -GAMMA * dtv / NTOT, tb[:],
                                               AO.mult, AO.add)
                nc.vector.scalar_tensor_tensor(scd_new[0:1, 1:2],
                                               scd[0:1, 0:1], -dtv,
                                               scd[0:1, 1:2], AO.mult, AO.add)
                scd = scd_new
                if t < T - 1:
                    agout_t = agout_next

                # ---- bias for step t+1: ceff = b1t[:,t+1] + w1x * d_{t+1} ----
                if t < T - 1:
                    pce = pssm.tile([128, 1], dt32, tag="sm")
                    nc.tensor.matmul(pce[:], C["w1xf"][:], scd[0:1, 1:2],
                                     start=True, stop=True)
                    ceff_t = tmpp.tile([128, 1], dt32, tag="ceff")
                    nc.vector.tensor_tensor(ceff_t[:], pce[:],
                                            C["b1t"][:, t + 1:t + 2], AO.add)

            if rep_ctx is not None:
                rep_ctx.__exit__(None, None, None)

            # ---- epilogue: add global scalars back, write out ----
            pcc = pssm.tile([128, 1], dt32, tag="sm")
            nc.tensor.matmul(pcc[:], r(C["onesr"][:]), r(scd[0:1, 0:1]),
                             start=True, stop=True)
            ccol = tmpp.tile([128, 1], dt32, tag="dcol")
            nc.vector.tensor_copy(ccol[:], pcc[:])
            pdd = pssm.tile([128, 1], dt32, tag="sm")
            nc.tensor.matmul(pdd[:], r(C["onesr"][:]), r(scd[0:1, 1:2]),
                             start=True, stop=True)
            dcol2 = tmpp.tile([128, 1], dt32, tag="dcol")
            nc.vector.tensor_copy(dcol2[:], pdd[:])
            yo = tmpp.tile([128, nch], dt32, tag="yo")
            nc.vector.tensor_scalar(yo[:], ut[:], ccol[:, 0:1], None, AO.add)
            xo = tmpp.tile([128, nch], dt32, tag="xo")
            nc.vector.tensor_scalar(xo[:], xt[:], dcol2[:, 0:1], float(CG),
                                    AO.add, AO.mult)
            nc.sync.dma_start(yout_d[:], yo[:])
            nc.sync.dma_start(xout_d[:], xo[:])

    nc.compile()
    return nc


def kernel(x, dw, y_params, z_params):
    from concourse.bass_utils import run_bass_kernel_spmd

    x = np.asarray(x, dtype=np.float32)
    dw = np.asarray(dw, dtype=np.float32)
    yp = [(np.asarray(w, np.float32), np.asarray(b, np.float32))
          for (w, b) in y_params]
    zp = [(np.asarray(w, np.float32), np.asarray(b, np.float32))
          for (w, b) in z_params]

    N, T = dw.shape[0], dw.shape[1]
    n_loc = N // N_CORES
    nch = n_loc // 128
    dtv = float(MATURITY) / T

    b3y = float(yp[2][1][0])
    b3z = float(zp[2][1][0])

    key = (T, n_loc, os.environ.get("NOCOLL", "0"), os.environ.get("REPDEV", "0"), os.environ.get("PSUM_GR", "1024"))
    if key not in _BUILD_CACHE:
        _BUILD_CACHE[key] = _build(T, n_loc, b3y, b3z, N)
    nc = _BUILD_CACHE[key]

    # ---- constants (shared across cores) ----
    w1z, b1z = zp[0]          # [2,128], [128]
    W2z, b2z = zp[1]          # [128,128], [128]
    w3z, _ = zp[2]            # [128,1]
    w1y, b1y = yp[0]          # [1,64], [64]
    W2y, b2y = yp[1]
    w3y, _ = yp[2]

    b1t = (b1z[None, :] + w1z[0][None, :] * (np.arange(T, dtype=np.float32)[:, None] * dtv)).T
    b1t = np.ascontiguousarray(b1t, dtype=np.float32)   # [128, T]

    Amat = np.zeros((10, 2), dtype=np.float32)
    Amat[0, 0] = 1.0 - GAMMA * dtv
    Amat[1, 0] = -CX * dtv
    Amat[2:, 0] = -GAMMA * dtv / N
    Amat[0, 1] = -dtv
    Amat[1, 1] = 1.0

    const_map = dict(
        w1x=np.ascontiguousarray(w1z[1:2, :]),
        w1xf=np.ascontiguousarray(w1z[1:2, :]),
        b1t=b1t,
        W2z=W2z,
        w3z=np.repeat(w3z.reshape(128, 1), 2, axis=1),
        b2z=b2z.reshape(128, 1),
        w1y=w1y.reshape(1, 64),
        b1y=b1y.reshape(64, 1),
        W2y=W2y,
        b2y=b2y.reshape(64, 1),
        w3y=np.repeat(w3y.reshape(64, 1), 2, axis=1),
        Amat=Amat,
        onesc=np.ones((128, 1), np.float32),
        onesr=np.ones((1, 128), np.float32),
        ident=np.eye(128, dtype=np.float32),
    )

    in_maps = []
    for c in range(N_CORES):
        sl = slice(c * n_loc, (c + 1) * n_loc)
        x_loc = x[sl, 0]                          # [n_loc]
        dw_loc = dw[sl, :, 0]                     # [n_loc, T]
        xcol = np.ascontiguousarray(x_loc.reshape(nch, 128).T)      # [128,nch]
        dwp = np.ascontiguousarray(
            dw_loc.reshape(nch, 128, T).transpose(1, 2, 0).reshape(128, T * nch))
        m = dict(const_map)
        m["xrow_in"] = x_loc.reshape(1, n_loc)
        m["xcol_in"] = xcol
        m["dwcol"] = dwp
        m["sdwcol"] = SIGMA * dwp
        in_maps.append(m)

    global _LAST_IN_MAPS
    _LAST_IN_MAPS = in_maps
    res = run_bass_kernel_spmd(nc, in_maps, list(range(N_CORES)))

    y_full = np.empty((N, 1), np.float32)
    x_full = np.empty((N, 1), np.float32)
    for c in range(N_CORES):
        sl = slice(c * n_loc, (c + 1) * n_loc)
        y_full[sl, 0] = res.results[c]["yout"].T.reshape(n_loc)
        x_full[sl, 0] = res.results[c]["xout"].T.reshape(n_loc)

    return (y_full, x_full)


# revision 27
# speedup vs baseline: 1.0345x; 1.0345x over previous
"""Trainium2 Bass kernel for nn_FBSDEModel_MFG (mean-field FBSDE simulation).

Math (reference semantics, CA=1):
    y0 = MLP_y(x)                        # 1->64->64->1, relu
    per step t (dt = 1/T):
      m_t    = mean(y_t)  over ALL samples          <-- only cross-core term
      y_diff = MLP_z([t*dt, x_t])                   # 2->128->128->1, relu
      x_{t+1} = x_t - y_t*dt + SIGMA*dw_t
      y_{t+1} = y_t - (CX*x_t + GAMMA*m_t)*dt + y_diff*dw_t
    out: (y_T, CG*x_T)

Decomposition used on device (exact algebra): y = u + c, x = xl + d with
c,d global scalars identical on every core:
      u_{t+1}  = u_t - CX*dt*xl_t + y_diff*dw_t          (local)
      xl_{t+1} = xl_t - dt*u_t + SIGMA*dw_t              (local)
      c_{t+1}  = (1-GAMMA*dt)c_t - CX*dt*d_t - (GAMMA*dt/N)*Sg_t
      d_{t+1}  = d_t - dt*c_t
      Sg_t     = sum over ALL samples of u_t   <-- AllGather of per-core sums
      MLP_z input x_t = xl_t + d_t  (d folded in as a shift before layer 1)

Sharding: pure data-parallel over 8 cores, 2048 samples each. The per-step
scalar AllGather (4B/core) is the only collective; it has ~1 full step of
compute to hide under.

Layouts: z-net activations are feature-major [feat, sample]; per-sample
state lives as [128, n_chunk] columns (sample s = chunk*128 + partition),
which is exactly the layout the chunked output-layer matmuls produce.
"""

import os
import sys

sys.path.insert(0, "/opt/trn_rl_repo")

import numpy as np

# model constants (must match reference.py)
CA = 1.0
CX = 0.1
GAMMA = 0.5
SIGMA = 0.7
CG = 1.0
MATURITY = 1.0

N_CORES = 8

_BUILD_CACHE = {}
_LAST_IN_MAPS = None


def _build(T, n_loc, b3y, b3z, NTOT):
    """Build + compile the SPMD bass program. Parametric in T and n_loc."""
    from concourse import bacc, mybir
    import concourse.tile as tile

    dt32 = mybir.dt.float32
    dt32r = mybir.dt.float32r
    AO = mybir.AluOpType
    Relu = mybir.ActivationFunctionType.Relu

    dtv = float(MATURITY) / T
    nch = n_loc // 128          # state columns (16 for n_loc=2048)
    n_free = n_loc              # z-net free dim per step
    nt512 = n_free // 512       # 512-wide matmul tiles (4)

    _rep = int(os.environ.get("REPDEV", "0")) > 1
    GR = int(os.environ.get("PSUM_GR", "1024"))
    _xb = 2 if not _rep else 4
    _sb = 3 if not _rep else 5
    nc = bacc.Bacc("TRN2", target_bir_lowering=False, debug=False,
                   num_devices=N_CORES)

    # ---- DRAM I/O ----
    D = {}
    def din(name, shape, dt=dt32):
        D[name] = nc.dram_tensor(name, shape, dt, kind="ExternalInput")
        return D[name]

    din("xrow_in", [1, n_loc], dt32r)
    din("xcol_in", [128, nch])
    din("dwcol", [128, T * nch])
    din("sdwcol", [128, T * nch])
    din("w1x", [1, 128], dt32r)
    din("w1xf", [1, 128])
    din("b1t", [128, T])
    din("W2z", [128, 128], dt32r)
    din("w3z", [128, 2], dt32r)
    din("b2z", [128, 1])
    din("w1y", [1, 64], dt32r)
    din("b1y", [64, 1])
    din("W2y", [64, 64], dt32r)
    din("b2y", [64, 1])
    din("w3y", [64, 2], dt32r)
    din("Amat", [10, 2])
    din("onesc", [128, 1])
    din("onesr", [1, 128])
    din("ident", [128, 128])
    yout_d = nc.dram_tensor("yout", [128, nch], dt32, kind="ExternalOutput")
    xout_d = nc.dram_tensor("xout", [128, nch], dt32, kind="ExternalOutput")

    r = lambda ap: ap

    with tile.TileContext(nc) as tc:
        with (
            tc.tile_pool(name="const", bufs=1) as cpool,
            tc.tile_pool(name="state", bufs=_sb) as spool,
            tc.tile_pool(name="hpool", bufs=2 if not _rep else 4) as hpool,
            tc.tile_pool(name="xrowp", bufs=_xb) as xrowp,
            tc.tile_pool(name="tmp", bufs=_sb) as tmpp,
            tc.tile_pool(name="psmm", bufs=(3 if GR == 1024 else 6), space="PSUM") as psmm,
            tc.tile_pool(name="pssm", bufs=2, space="PSUM") as pssm,
            tc.tile_pool(name="dram", bufs=4, space="DRAM") as dpool,
        ):
            # ---- load constants ----
            C = {}
            for name, shape in [
                ("w1x", [1, 128]), ("w1xf", [1, 128]), ("b1t", [128, T]), ("W2z", [128, 128]),
                ("w3z", [128, 2]), ("b2z", [128, 1]), ("w1y", [1, 64]),
                ("b1y", [64, 1]), ("W2y", [64, 64]), ("b2y", [64, 1]),
                ("w3y", [64, 2]), ("Amat", [10, 2]), ("onesc", [128, 1]),
                ("onesr", [1, 128]), ("ident", [128, 128]),
                ("dwcol", [128, T * nch]), ("sdwcol", [128, T * nch]),
            ]:
                cdt = dt32r if name in ("w1x", "W2z", "w3z", "w1y", "W2y", "w3y") else dt32
                C[name] = cpool.tile(shape, cdt, tag=f"c_{name}", name=f"c_{name}")
                nc.sync.dma_start(C[name][:], D[name][:])

            # scalar state (c, d) as a [1,2] row on partition 0
            scd = tmpp.tile([1, 2], dt32, tag="scd")
            nc.vector.memset(scd[:], 0.0)

            # ---- initial x row + column state ----
            xrow0 = xrowp.tile([1, n_loc], dt32r, tag="xrow")
            nc.sync.dma_start(xrow0[:], D["xrow_in"][:])
            xt = spool.tile([128, nch], dt32, tag="xt")
            nc.sync.dma_start(xt[:], D["xcol_in"][:])

            # ---- y-net: u0 = MLP_y(x) ----
            h1y = tmpp.tile([64, n_free], dt32r, tag="h1y")
            h2y = tmpp.tile([64, n_free], dt32r, tag="h2y")
            for h in range(n_free // GR):
                py = psmm.tile([64, GR], dt32, tag="mm")
                for j in range(GR // 512):
                    s = h * GR + j * 512
                    nc.tensor.matmul(py[:, j * 512:(j + 1) * 512],
                                     r(C["w1y"][:]), r(xrow0[0:1, s:s + 512]),
                                     start=True, stop=True)
                nc.scalar.activation(h1y[:, h * GR:(h + 1) * GR], py[:],
                                     Relu, bias=C["b1y"][:, 0:1])
            for h in range(n_free // GR):
                py = psmm.tile([64, GR], dt32, tag="mm")
                for j in range(GR // 512):
                    s = h * GR + j * 512
                    nc.tensor.matmul(py[:, j * 512:(j + 1) * 512],
                                     r(C["W2y"][:]), r(h1y[:, s:s + 512]),
                                     start=True, stop=True)
                nc.scalar.activation(h2y[:, h * GR:(h + 1) * GR], py[:],
                                     Relu, bias=C["b2y"][:, 0:1])
            py0 = pssm.tile([128, 2 * nch], dt32, tag="sm")
            for k in range(nch):
                nc.tensor.matmul(py0[:, 2 * k:2 * k + 2],
                                 r(h2y[:, k * 128:(k + 1) * 128]),
                                 r(C["w3y"][:]), start=True, stop=True)
            ut = spool.tile([128, nch], dt32, tag="ut")
            uacc = tmpp.tile([128, 1], dt32, tag="uacc")
            nc.vector.tensor_scalar(ut[:], py0[:, 0:2 * nch:2], float(b3y),
                                    0.0, AO.add, AO.add, accum_out=uacc[:])

            NOCOLL = os.environ.get("NOCOLL", "0") == "1"
            # ---- S_0 partial sum + AllGather launch ----
            def launch_ag(uacc_tile):
                ps = pssm.tile([1, 1], dt32, tag="sm")
                nc.tensor.matmul(ps[:], r(C["onesc"][:]), r(uacc_tile[:]),
                                 start=True, stop=True)
                ssb = tmpp.tile([1, 8], dt32, tag="ssb")
                nc.vector.tensor_copy(ssb[0:1, 0:1], ps[:])
                agin = dpool.tile([1, 8], dt32, tag="agin")
                agout = dpool.tile([8, 8], dt32, tag="agout")
                nc.gpsimd.dma_start(agin[0:1, 0:1], ssb[0:1, 0:1])
                if NOCOLL:
                    nc.sync.dma_start(agout[0:1, :], agin[0:1, :])
                else:
                    nc.gpsimd.collective_compute(
                        "AllGather", AO.bypass,
                        replica_groups=[list(range(N_CORES))],
                        ins=[agin.opt()], outs=[agout.opt()],
                    )
                return agout

            agout_t = launch_ag(uacc)

            REPDEV = int(os.environ.get("REPDEV", "0"))
            rep_ctx = tc.For_i(0, REPDEV, 1) if REPDEV > 1 else None
            if rep_ctx is not None:
                rep_ctx.__enter__()

            xrow_t = xrow0
            ceff_t = None           # step-0 bias handled via b1t directly
            for t in range(T):
                dwt = C["dwcol"][:, t * nch:(t + 1) * nch]
                sdwt = C["sdwcol"][:, t * nch:(t + 1) * nch]

                # ---- x-state advance (needs only step t-1 data) ----
                t3 = tmpp.tile([128, nch], dt32, tag="t3")
                nc.vector.scalar_tensor_tensor(t3[:], ut[:], -dtv, sdwt,
                                               AO.mult, AO.add)
                xt_new = spool.tile([128, nch], dt32, tag="xt")
                nc.vector.tensor_add(xt_new[:], xt[:], t3[:])

                # ---- bridge for step t+1 (off critical path) ----
                if t < T - 1:
                    pT = pssm.tile([nch, 128], dt32, tag="sm")
                    nc.tensor.transpose(pT[:], xt_new[:], C["ident"][:])
                    sT = tmpp.tile([nch, 128], dt32r, tag="sT")
                    nc.vector.tensor_copy(sT[:], pT[:])
                    xrow_next = xrowp.tile([1, n_loc], dt32r, tag="xrow")
                    nc.sync.dma_start(xrow_next[0:1, :], sT[:, :])

                # ---- z-net layer 1 (rank-1) + relu (bias carries d_t) ----
                bias1 = C["b1t"][:, t:t + 1] if ceff_t is None else ceff_t[:, 0:1]
                h1 = hpool.tile([128, n_free], dt32r, tag="h1")
                for h in range(n_free // GR):
                    p1 = psmm.tile([128, GR], dt32, tag="mm")
                    for j in range(GR // 512):
                        s = h * GR + j * 512
                        nc.tensor.matmul(p1[:, j * 512:(j + 1) * 512],
                                         r(C["w1x"][:]),
                                         r(xrow_t[0:1, s:s + 512]),
                                         start=True, stop=True)
                    nc.scalar.activation(h1[:, h * GR:(h + 1) * GR],
                                         p1[:], Relu, bias=bias1)

                # ---- z-net layer 2 + relu (split ACT/DVE) ----
                h2 = hpool.tile([128, n_free], dt32r, tag="h2")
                for h in range(n_free // GR):
                    p2 = psmm.tile([128, GR], dt32, tag="mm")
                    for j in range(GR // 512):
                        s = h * GR + j * 512
                        nc.tensor.matmul(p2[:, j * 512:(j + 1) * 512],
                                         r(C["W2z"][:]), r(h1[:, s:s + 512]),
                                         start=True, stop=True)
                    if h >= (n_free // GR) // 2:
                        nc.scalar.activation(h2[:, h * GR:(h + 1) * GR],
                                             p2[:], Relu, bias=C["b2z"][:, 0:1])
                    else:
                        nc.vector.tensor_scalar(h2[:, h * GR:(h + 1) * GR],
                                                p2[:], C["b2z"][:, 0:1], 0.0,
                                                AO.add, AO.max)

                # ---- z-net layer 3, chunked into state layout ----
                pyd = pssm.tile([128, 2 * nch], dt32, tag="sm")
                for k in range(nch):
                    nc.tensor.matmul(pyd[:, 2 * k:2 * k + 2],
                                     r(h2[:, k * 128:(k + 1) * 128]),
                                     r(C["w3z"][:]), start=True, stop=True)

                # ---- u-state update ----
                q = tmpp.tile([128, nch], dt32, tag="q")
                nc.vector.scalar_tensor_tensor(q[:], pyd[:, 0:2 * nch:2],
                                               float(b3z), dwt, AO.add,
                                               AO.mult)
                t2 = tmpp.tile([128, nch], dt32, tag="t2")
                nc.vector.scalar_tensor_tensor(t2[:], xt[:], -CX * dtv, ut[:],
                                               AO.mult, AO.add)
                ut_new = spool.tile([128, nch], dt32, tag="ut")
                uacc = tmpp.tile([128, 1], dt32, tag="uacc")
                nc.vector.scalar_tensor_tensor(ut_new[:], t2[:], 1.0, q[:],
                                               AO.mult, AO.add,
                                               accum_out=uacc[:])
                ut, xt = ut_new, xt_new
                if t < T - 1:
                    xrow_t = xrow_next

                # ---- consume AllGather_t BEFORE launching the next one, so
                # the result-read DMA is not queued behind the next launch on
                # the Pool engine ----
                sgrow = tmpp.tile([1, 8], dt32, tag="sgrow")
                nc.gpsimd.dma_start(sgrow[0:1, :], agout_t[0:8, 0:1])
                sg1 = tmpp.tile([1, 1], dt32, tag="sg1")
                nc.vector.tensor_reduce(sg1[:], sgrow[:], mybir.AxisListType.X,
                                        AO.add)
                scd_new = tmpp.tile([1, 2], dt32, tag="scd")
                ta = tmpp.tile([1, 1], dt32, tag="ta")
                nc.vector.tensor_scalar(ta[:], scd[0:1, 0:1],
                                        1.0 - GAMMA * dtv, None, AO.mult)
                tb = tmpp.tile([1, 1], dt32, tag="tb")
                nc.vector.scalar_tensor_tensor(tb[:], scd[0:1, 1:2], -CX * dtv,
                                               ta[:], AO.mult, AO.add)
                nc.vector.scalar_tensor_tensor(scd_new[0:1, 0:1], sg1[:],
                                               -GAMMA * dtv / NTOT, tb[:],
                                               AO.mult, AO.add)
                nc.vector.scalar_tensor_tensor(scd_new[0:1, 1:2],
                                               scd[0:1, 0:1], -dtv,
                                               scd[0:1, 1:2], AO.mult, AO.add)
                scd = scd_new

                # ---- bias for step t+1: ceff = b1t[:,t+1] + w1x * d_{t+1} ----
                if t < T - 1:
                    pce = pssm.tile([128, 1], dt32, tag="sm")
                    nc.tensor.matmul(pce[:], C["w1xf"][:], scd[0:1, 1:2],
                                     start=True, stop=True)
                    ceff_t = tmpp.tile([128, 1], dt32, tag="ceff")
                    nc.vector.tensor_tensor(ceff_t[:], pce[:],
                                            C["b1t"][:, t + 1:t + 2], AO.add)

                # ---- launch AllGather of S_{t+1} ----
                if t < T - 1:
                    agout_t = launch_ag(uacc)

            if rep_ctx is not None:
                rep_ctx.__exit__(None, None, None)

            # ---- epilogue: add global scalars back, write out ----
            pcc = pssm.tile([128, 1], dt32, tag="sm")
            nc.tensor.matmul(pcc[:], r(C["onesr"][:]), r(scd[0:1, 0:1]),
                             start=True, stop=True)
            ccol = tmpp.tile([128, 1], dt32, tag="dcol")
            nc.vector.tensor_copy(ccol[:], pcc[:])
            pdd = pssm.tile([128, 1], dt32, tag="sm")
            nc.tensor.matmul(pdd[:], r(C["onesr"][:]), r(scd[0:1, 1:2]),
                             start=True, stop=True)
            dcol2 = tmpp.tile([128, 1], dt32, tag="dcol")
            nc.vector.tensor_copy(dcol2[:], pdd[:])
            yo = tmpp.tile([128, nch], dt32, tag="yo")
            nc.vector.tensor_scalar(yo[:], ut[:], ccol[:, 0:1], None, AO.add)
            xo = tmpp.tile([128, nch], dt32, tag="xo")
            nc.vector.tensor_scalar(xo[:], xt[:], dcol2[:, 0:1], float(CG),
                                    AO.add, AO.mult)
            nc.sync.dma_start(yout_d[:], yo[:])
            nc.sync.dma_start(xout_d[:], xo[:])

    nc.compile()
    return nc


def kernel(x, dw, y_params, z_params):
    from concourse.bass_utils import run_bass_kernel_spmd

    x = np.asarray(x, dtype=np.float32)
    dw = np.asarray(dw, dtype=np.float32)
    yp = [(np.asarray(w, np.float32), np.asarray(b, np.float32))
          for (w, b) in y_params]
    zp = [(np.asarray(w, np.float32), np.asarray(b, np.float32))
          for (w, b) in z_params]

    N, T = dw.shape[0], dw.shape[1]
    n_loc = N // N_CORES
    nch = n_loc // 128
    dtv = float(MATURITY) / T

    b3y = float(yp[2][1][0])
    b3z = float(zp[2][1][0])

    key = (T, n_loc, os.environ.get("NOCOLL", "0"), os.environ.get("REPDEV", "0"), os.environ.get("PSUM_GR", "1024"))
    if key not in _BUILD_CACHE:
        _BUILD_CACHE[key] = _build(T, n_loc, b3y, b3z, N)
    nc = _BUILD_CACHE[key]

    # ---- constants (shared across cores) ----
    w1z, b1z = zp[0]          # [2,128], [128]
    W2z, b2z = zp[1]          # [128,128], [128]
    w3z, _ = zp[2]            # [128,1]
    w1y, b1y = yp[0]          # [1,64], [64]
    W2y, b2y = yp[1]
    w3y, _ = yp[2]

    b1t = (b1z[None, :] + w1z[0][None, :] * (np.arange(T, dtype=np.float32)[:, None] * dtv)).T
    b1t = np.ascontiguousarray(b1t, dtype=np.float32)   # [128, T]

    Amat = np.zeros((10, 2), dtype=np.float32)
    Amat[0, 0] = 1.0 - GAMMA * dtv
    Amat[1, 0] = -CX * dtv
    Amat[2:, 0] = -GAMMA * dtv / N
    Amat[0, 1] = -dtv
    Amat[1, 1] = 1.0

    const_map = dict(
        w1x=np.ascontiguousarray(w1z[1:2, :]),
        w1xf=np.ascontiguousarray(w1z[1:2, :]),
        b1t=b1t,
        W2z=W2z,
        w3z=np.repeat(w3z.reshape(128, 1), 2, axis=1),
        b2z=b2z.reshape(128, 1),
        w1y=w1y.reshape(1, 64),
        b1y=b1y.reshape(64, 1),
        W2y=W2y,
        b2y=b2y.reshape(64, 1),
        w3y=np.repeat(w3y.reshape(64, 1), 2, axis=1),
        Amat=Amat,
        onesc=np.ones((128, 1), np.float32),
        onesr=np.ones((1, 128), np.float32),
        ident=np.eye(128, dtype=np.float32),
    )

    in_maps = []
    for c in range(N_CORES):
        sl = slice(c * n_loc, (c + 1) * n_loc)
        x_loc = x[sl, 0]                          # [n_loc]
        dw_loc = dw[sl, :, 0]                     # [n_loc, T]
        xcol = np.ascontiguousarray(x_loc.reshape(nch, 128).T)      # [128,nch]
        dwp = np.ascontiguousarray(
            dw_loc.reshape(nch, 128, T).transpose(1, 2, 0).reshape(128, T * nch))
        m = dict(const_map)
        m["xrow_in"] = x_loc.reshape(1, n_loc)
        m["xcol_in"] = xcol
        m["dwcol"] = dwp
        m["sdwcol"] = SIGMA * dwp
        in_maps.append(m)

    global _LAST_IN_MAPS
    _LAST_IN_MAPS = in_maps
    res = run_bass_kernel_spmd(nc, in_maps, list(range(N_CORES)))

    y_full = np.empty((N, 1), np.float32)
    x_full = np.empty((N, 1), np.float32)
    for c in range(N_CORES):
        sl = slice(c * n_loc, (c + 1) * n_loc)
        y_full[sl, 0] = res.results[c]["yout"].T.reshape(n_loc)
        x_full[sl, 0] = res.results[c]["xout"].T.reshape(n_loc)

    return (y_full, x_full)


# revision 29
# speedup vs baseline: 1.0399x; 1.0052x over previous
"""Trainium2 Bass kernel for nn_FBSDEModel_MFG (mean-field FBSDE simulation).

Math (reference semantics, CA=1):
    y0 = MLP_y(x)                        # 1->64->64->1, relu
    per step t (dt = 1/T):
      m_t    = mean(y_t)  over ALL samples          <-- only cross-core term
      y_diff = MLP_z([t*dt, x_t])                   # 2->128->128->1, relu
      x_{t+1} = x_t - y_t*dt + SIGMA*dw_t
      y_{t+1} = y_t - (CX*x_t + GAMMA*m_t)*dt + y_diff*dw_t
    out: (y_T, CG*x_T)

Decomposition used on device (exact algebra): y = u + c, x = xl + d with
c,d global scalars identical on every core:
      u_{t+1}  = u_t - CX*dt*xl_t + y_diff*dw_t          (local)
      xl_{t+1} = xl_t - dt*u_t + SIGMA*dw_t              (local)
      c_{t+1}  = (1-GAMMA*dt)c_t - CX*dt*d_t - (GAMMA*dt/N)*Sg_t
      d_{t+1}  = d_t - dt*c_t
      Sg_t     = sum over ALL samples of u_t   <-- AllGather of per-core sums
      MLP_z input x_t = xl_t + d_t  (d folded in as a shift before layer 1)

Sharding: pure data-parallel over 8 cores, 2048 samples each. The per-step
scalar AllGather (4B/core) is the only collective; it has ~1 full step of
compute to hide under.

Layouts: z-net activations are feature-major [feat, sample]; per-sample
state lives as [128, n_chunk] columns (sample s = chunk*128 + partition),
which is exactly the layout the chunked output-layer matmuls produce.
"""

import os
import sys

sys.path.insert(0, "/opt/trn_rl_repo")

import numpy as np

# model constants (must match reference.py)
CA = 1.0
CX = 0.1
GAMMA = 0.5
SIGMA = 0.7
CG = 1.0
MATURITY = 1.0

N_CORES = 8

_BUILD_CACHE = {}
_LAST_IN_MAPS = None


def _build(T, n_loc, b3y, b3z, NTOT):
    """Build + compile the SPMD bass program. Parametric in T and n_loc."""
    from concourse import bacc, mybir
    import concourse.tile as tile

    dt32 = mybir.dt.float32
    dt32r = mybir.dt.float32r
    AO = mybir.AluOpType
    Relu = mybir.ActivationFunctionType.Relu

    dtv = float(MATURITY) / T
    nch = n_loc // 128          # state columns (16 for n_loc=2048)
    n_free = n_loc              # z-net free dim per step
    nt512 = n_free // 512       # 512-wide matmul tiles (4)

    _rep = int(os.environ.get("REPDEV", "0")) > 1
    GR = int(os.environ.get("PSUM_GR", "1024"))
    _xb = 2 if not _rep else 4
    _sb = 3 if not _rep else 5
    nc = bacc.Bacc("TRN2", target_bir_lowering=False, debug=False,
                   num_devices=N_CORES)

    # ---- DRAM I/O ----
    D = {}
    def din(name, shape, dt=dt32):
        D[name] = nc.dram_tensor(name, shape, dt, kind="ExternalInput")
        return D[name]

    din("xrow_in", [1, n_loc], dt32r)
    din("xcol_in", [128, nch])
    din("dwcol", [128, T * nch])
    din("sdwcol", [128, T * nch])
    din("w1x", [1, 128], dt32r)
    din("w1xf", [1, 128])
    din("b1t", [128, T])
    din("W2z", [128, 128], dt32r)
    din("w3z", [128, 2], dt32r)
    din("b2z", [128, 1])
    din("w1y", [1, 64], dt32r)
    din("b1y", [64, 1])
    din("W2y", [64, 64], dt32r)
    din("b2y", [64, 1])
    din("w3y", [64, 2], dt32r)
    din("Amat", [10, 2])
    din("onesc", [128, 1])
    din("onesr", [1, 128])
    din("ident", [128, 128])
    yout_d = nc.dram_tensor("yout", [128, nch], dt32, kind="ExternalOutput")
    xout_d = nc.dram_tensor("xout", [128, nch], dt32, kind="ExternalOutput")

    r = lambda ap: ap

    with tile.TileContext(nc) as tc:
        with (
            tc.tile_pool(name="const", bufs=1) as cpool,
            tc.tile_pool(name="state", bufs=_sb) as spool,
            tc.tile_pool(name="hpool", bufs=2 if not _rep else 4) as hpool,
            tc.tile_pool(name="xrowp", bufs=_xb) as xrowp,
            tc.tile_pool(name="tmp", bufs=_sb) as tmpp,
            tc.tile_pool(name="psmm", bufs=(3 if GR == 1024 else 6), space="PSUM") as psmm,
            tc.tile_pool(name="pssm", bufs=2, space="PSUM") as pssm,
            tc.tile_pool(name="dram", bufs=4, space="DRAM") as dpool,
        ):
            # ---- load constants ----
            C = {}
            for name, shape in [
                ("w1x", [1, 128]), ("w1xf", [1, 128]), ("b1t", [128, T]), ("W2z", [128, 128]),
                ("w3z", [128, 2]), ("b2z", [128, 1]), ("w1y", [1, 64]),
                ("b1y", [64, 1]), ("W2y", [64, 64]), ("b2y", [64, 1]),
                ("w3y", [64, 2]), ("Amat", [10, 2]), ("onesc", [128, 1]),
                ("onesr", [1, 128]), ("ident", [128, 128]),
                ("dwcol", [128, T * nch]), ("sdwcol", [128, T * nch]),
            ]:
                cdt = dt32r if name in ("w1x", "W2z", "w3z", "w1y", "W2y", "w3y") else dt32
                C[name] = cpool.tile(shape, cdt, tag=f"c_{name}", name=f"c_{name}")
                nc.sync.dma_start(C[name][:], D[name][:])

            # scalar state (c, d) as a [1,2] row on partition 0
            scd = tmpp.tile([1, 2], dt32, tag="scd")
            nc.vector.memset(scd[:], 0.0)

            # ---- initial x row + column state ----
            xrow0 = xrowp.tile([1, n_loc], dt32r, tag="xrow")
            nc.sync.dma_start(xrow0[:], D["xrow_in"][:])
            xt = spool.tile([128, nch], dt32, tag="xt")
            nc.sync.dma_start(xt[:], D["xcol_in"][:])

            # ---- y-net: u0 = MLP_y(x) ----
            h1y = tmpp.tile([64, n_free], dt32r, tag="h1y")
            h2y = tmpp.tile([64, n_free], dt32r, tag="h2y")
            for h in range(n_free // GR):
                py = psmm.tile([64, GR], dt32, tag="mm")
                for j in range(GR // 512):
                    s = h * GR + j * 512
                    nc.tensor.matmul(py[:, j * 512:(j + 1) * 512],
                                     r(C["w1y"][:]), r(xrow0[0:1, s:s + 512]),
                                     start=True, stop=True)
                nc.scalar.activation(h1y[:, h * GR:(h + 1) * GR], py[:],
                                     Relu, bias=C["b1y"][:, 0:1])
            for h in range(n_free // GR):
                py = psmm.tile([64, GR], dt32, tag="mm")
                for j in range(GR // 512):
                    s = h * GR + j * 512
                    nc.tensor.matmul(py[:, j * 512:(j + 1) * 512],
                                     r(C["W2y"][:]), r(h1y[:, s:s + 512]),
                                     start=True, stop=True)
                nc.scalar.activation(h2y[:, h * GR:(h + 1) * GR], py[:],
                                     Relu, bias=C["b2y"][:, 0:1])
            py0 = pssm.tile([128, 2 * nch], dt32, tag="sm")
            for k in range(nch):
                nc.tensor.matmul(py0[:, 2 * k:2 * k + 2],
                                 r(h2y[:, k * 128:(k + 1) * 128]),
                                 r(C["w3y"][:]), start=True, stop=True)
            ut = spool.tile([128, nch], dt32, tag="ut")
            uacc = tmpp.tile([128, 1], dt32, tag="uacc")
            nc.vector.tensor_scalar(ut[:], py0[:, 0:2 * nch:2], float(b3y),
                                    0.0, AO.add, AO.add, accum_out=uacc[:])

            NOCOLL = os.environ.get("NOCOLL", "0") == "1"
            # ---- S_0 partial sum + AllGather launch ----
            def launch_ag(uacc_tile):
                ps = pssm.tile([1, 1], dt32, tag="sm")
                nc.tensor.matmul(ps[:], r(C["onesc"][:]), r(uacc_tile[:]),
                                 start=True, stop=True)
                ssb = tmpp.tile([1, 8], dt32, tag="ssb")
                nc.vector.tensor_copy(ssb[0:1, 0:1], ps[:])
                agin = dpool.tile([1, 8], dt32, tag="agin")
                agout = dpool.tile([8, 8], dt32, tag="agout")
                nc.gpsimd.dma_start(agin[0:1, 0:1], ssb[0:1, 0:1])
                if NOCOLL:
                    nc.sync.dma_start(agout[0:1, :], agin[0:1, :])
                else:
                    nc.gpsimd.collective_compute(
                        "AllGather", AO.bypass,
                        replica_groups=[list(range(N_CORES))],
                        ins=[agin.opt()], outs=[agout.opt()],
                    )
                return agout

            agout_t = launch_ag(uacc)

            REPDEV = int(os.environ.get("REPDEV", "0"))
            rep_ctx = tc.For_i(0, REPDEV, 1) if REPDEV > 1 else None
            if rep_ctx is not None:
                rep_ctx.__enter__()

            xrow_t = xrow0
            ceff_t = None           # step-0 bias handled via b1t directly
            for t in range(T):
                dwt = C["dwcol"][:, t * nch:(t + 1) * nch]
                sdwt = C["sdwcol"][:, t * nch:(t + 1) * nch]

                # ---- x-state advance (needs only step t-1 data) ----
                t3 = tmpp.tile([128, nch], dt32, tag="t3")
                nc.vector.scalar_tensor_tensor(t3[:], ut[:], -dtv, sdwt,
                                               AO.mult, AO.add)
                xt_new = spool.tile([128, nch], dt32, tag="xt")
                nc.vector.tensor_add(xt_new[:], xt[:], t3[:])

                # ---- bridge for step t+1 (off critical path) ----
                if t < T - 1:
                    pT = pssm.tile([nch, 128], dt32, tag="sm")
                    nc.tensor.transpose(pT[:], xt_new[:], C["ident"][:])
                    sT = tmpp.tile([nch, 128], dt32r, tag="sT")
                    nc.vector.tensor_copy(sT[:], pT[:])
                    xrow_next = xrowp.tile([1, n_loc], dt32r, tag="xrow")
                    nc.sync.dma_start(xrow_next[0:1, :], sT[:, :])

                # ---- z-net layer 1 (rank-1) + relu (bias carries d_t) ----
                bias1 = C["b1t"][:, t:t + 1] if ceff_t is None else ceff_t[:, 0:1]
                h1 = hpool.tile([128, n_free], dt32r, tag="h1")
                for h in range(n_free // GR):
                    p1 = psmm.tile([128, GR], dt32, tag="mm")
                    for j in range(GR // 512):
                        s = h * GR + j * 512
                        nc.tensor.matmul(p1[:, j * 512:(j + 1) * 512],
                                         r(C["w1x"][:]),
                                         r(xrow_t[0:1, s:s + 512]),
                                         start=True, stop=True)
                    nc.scalar.activation(h1[:, h * GR:(h + 1) * GR],
                                         p1[:], Relu, bias=bias1)

                # ---- z-net layer 2 + relu (split ACT/DVE) ----
                h2 = hpool.tile([128, n_free], dt32r, tag="h2")
                for h in range(n_free // GR):
                    p2 = psmm.tile([128, GR], dt32, tag="mm")
                    for j in range(GR // 512):
                        s = h * GR + j * 512
                        nc.tensor.matmul(p2[:, j * 512:(j + 1) * 512],
                                         r(C["W2z"][:]), r(h1[:, s:s + 512]),
                                         start=True, stop=True)
                    if h >= (n_free // GR) // 2:
                        nc.scalar.activation(h2[:, h * GR:(h + 1) * GR],
                                             p2[:], Relu, bias=C["b2z"][:, 0:1])
                    else:
                        nc.vector.tensor_scalar(h2[:, h * GR:(h + 1) * GR],
                                                p2[:], C["b2z"][:, 0:1], 0.0,
                                                AO.add, AO.max)

                # ---- z-net layer 3, chunked into state layout ----
                pyd = pssm.tile([128, 2 * nch], dt32, tag="sm")
                for k in range(nch):
                    nc.tensor.matmul(pyd[:, 2 * k:2 * k + 2],
                                     r(h2[:, k * 128:(k + 1) * 128]),
                                     r(C["w3z"][:]), start=True, stop=True)

                # ---- u-state update ----
                q = tmpp.tile([128, nch], dt32, tag="q")
                nc.vector.scalar_tensor_tensor(q[:], pyd[:, 0:2 * nch:2],
                                               float(b3z), dwt, AO.add,
                                               AO.mult)
                t2 = tmpp.tile([128, nch], dt32, tag="t2")
                nc.vector.scalar_tensor_tensor(t2[:], xt[:], -CX * dtv, ut[:],
                                               AO.mult, AO.add)
                ut_new = spool.tile([128, nch], dt32, tag="ut")
                uacc = tmpp.tile([128, 1], dt32, tag="uacc")
                nc.vector.scalar_tensor_tensor(ut_new[:], t2[:], 1.0, q[:],
                                               AO.mult, AO.add,
                                               accum_out=uacc[:])
                ut, xt = ut_new, xt_new
                if t < T - 1:
                    xrow_t = xrow_next

                # ---- consume AllGather_t BEFORE launching the next one, so
                # the result-read DMA is not queued behind the next launch on
                # the Pool engine ----
                sgrow = tmpp.tile([1, 8], dt32, tag="sgrow")
                nc.gpsimd.dma_start(sgrow[0:1, :], agout_t[0:8, 0:1])
                sg1 = tmpp.tile([1, 1], dt32, tag="sg1")
                nc.vector.tensor_reduce(sg1[:], sgrow[:], mybir.AxisListType.X,
                                        AO.add)
                scd_new = tmpp.tile([1, 2], dt32, tag="scd")
                ta = tmpp.tile([1, 1], dt32, tag="ta")
                nc.vector.tensor_scalar(ta[:], scd[0:1, 0:1],
                                        1.0 - GAMMA * dtv, None, AO.mult)
                tb = tmpp.tile([1, 1], dt32, tag="tb")
                nc.vector.scalar_tensor_tensor(tb[:], scd[0:1, 1:2], -CX * dtv,
                                               ta[:], AO.mult, AO.add)
                nc.vector.scalar_tensor_tensor(scd_new[0:1, 0:1], sg1[:],
                                               -GAMMA * dtv / NTOT, tb[:],
                                               AO.mult, AO.add)
                nc.vector.scalar_tensor_tensor(scd_new[0:1, 1:2],
                                               scd[0:1, 0:1], -dtv,
                                               scd[0:1, 1:2], AO.mult, AO.add)
                scd = scd_new

                # ---- bias for step t+1: ceff = b1t[:,t+1] + w1x * d_{t+1} ----
                if t < T - 1:
                    pce = pssm.tile([128, 1], dt32, tag="sm")
                    nc.tensor.matmul(pce[:], C["w1xf"][:], scd[0:1, 1:2],
                                     start=True, stop=True)
                    ceff_t = tmpp.tile([128, 1], dt32, tag="ceff")
                    nc.vector.tensor_tensor(ceff_t[:], pce[:],
                                            C["b1t"][:, t + 1:t + 2], AO.add)

                # ---- launch AllGather of S_{t+1} ----
                if t < T - 1:
                    agout_t = launch_ag(uacc)

            if rep_ctx is not None:
                rep_ctx.__exit__(None, None, None)

            # ---- epilogue: add global scalars back, write out ----
            pcc = pssm.tile([128, 1], dt32, tag="sm")
            nc.tensor.matmul(pcc[:], r(C["onesr"][:]), r(scd[0:1, 0:1]),
                             start=True, stop=True)
            ccol = tmpp.tile([128, 1], dt32, tag="dcol")
            nc.vector.tensor_copy(ccol[:], pcc[:])
            pdd = pssm.tile([128, 1], dt32, tag="sm")
            nc.tensor.matmul(pdd[:], r(C["onesr"][:]), r(scd[0:1, 1:2]),
                             start=True, stop=True)
            dcol2 = tmpp.tile([128, 1], dt32, tag="dcol")
            nc.vector.tensor_copy(dcol2[:], pdd[:])
            yo = tmpp.tile([128, nch], dt32, tag="yo")
            nc.vector.tensor_scalar(yo[:], ut[:], ccol[:, 0:1], None, AO.add)
            xo = tmpp.tile([128, nch], dt32, tag="xo")
            nc.vector.tensor_scalar(xo[:], xt[:], dcol2[:, 0:1], float(CG),
                                    AO.add, AO.mult)
            nc.sync.dma_start(yout_d[:], yo[:])
            nc.sync.dma_start(xout_d[:], xo[:])

    nc.compile()
    return nc


def kernel(x, dw, y_params, z_params):
    from concourse.bass_utils import run_bass_kernel_spmd

    x = np.asarray(x, dtype=np.float32)
    dw = np.asarray(dw, dtype=np.float32)
    yp = [(np.asarray(w, np.float32), np.asarray(b, np.float32))
          for (w, b) in y_params]
    zp = [(np.asarray(w, np.float32), np.asarray(b, np.float32))
          for (w, b) in z_params]

    N, T = dw.shape[0], dw.shape[1]
    n_loc = N // N_CORES
    nch = n_loc // 128
    dtv = float(MATURITY) / T

    b3y = float(yp[2][1][0])
    b3z = float(zp[2][1][0])

    key = (T, n_loc, os.environ.get("NOCOLL", "0"), os.environ.get("REPDEV", "0"), os.environ.get("PSUM_GR", "1024"))
    if key not in _BUILD_CACHE:
        _BUILD_CACHE[key] = _build(T, n_loc, b3y, b3z, N)
    nc = _BUILD_CACHE[key]

    # ---- constants (shared across cores) ----
    w1z, b1z = zp[0]          # [2,128], [128]
    W2z, b2z = zp[1]          # [128,128], [128]
    w3z, _ = zp[2]            # [128,1]
    w1y, b1y = yp[0]          # [1,64], [64]
    W2y, b2y = yp[1]
    w3y, _ = yp[2]

    b1t = (b1z[None, :] + w1z[0][None, :] * (np.arange(T, dtype=np.float32)[:, None] * dtv)).T
    b1t = np.ascontiguousarray(b1t, dtype=np.float32)   # [128, T]

    Amat = np.zeros((10, 2), dtype=np.float32)
    Amat[0, 0] = 1.0 - GAMMA * dtv
    Amat[1, 0] = -CX * dtv
    Amat[2:, 0] = -GAMMA * dtv / N
    Amat[0, 1] = -dtv
    Amat[1, 1] = 1.0

    const_map = dict(
        w1x=np.ascontiguousarray(w1z[1:2, :]),
        w1xf=np.ascontiguousarray(w1z[1:2, :]),
        b1t=b1t,
        W2z=W2z,
        w3z=np.repeat(w3z.reshape(128, 1), 2, axis=1),
        b2z=b2z.reshape(128, 1),
        w1y=w1y.reshape(1, 64),
        b1y=b1y.reshape(64, 1),
        W2y=W2y,
        b2y=b2y.reshape(64, 1),
        w3y=np.repeat(w3y.reshape(64, 1), 2, axis=1),
        Amat=Amat,
        onesc=np.ones((128, 1), np.float32),
        onesr=np.ones((1, 128), np.float32),
        ident=np.eye(128, dtype=np.float32),
    )

    in_maps = []
    for c in range(N_CORES):
        sl = slice(c * n_loc, (c + 1) * n_loc)
        x_loc = x[sl, 0]                          # [n_loc]
        dw_loc = dw[sl, :, 0]                     # [n_loc, T]
        xcol = np.ascontiguousarray(x_loc.reshape(nch, 128).T)      # [128,nch]
        dwp = np.ascontiguousarray(
            dw_loc.reshape(nch, 128, T).transpose(1, 2, 0).reshape(128, T * nch))
        m = dict(const_map)
        m["xrow_in"] = x_loc.reshape(1, n_loc)
        m["xcol_in"] = xcol
        m["dwcol"] = dwp
        m["sdwcol"] = SIGMA * dwp
        in_maps.append(m)

    global _LAST_IN_MAPS
    _LAST_IN_MAPS = in_maps
    res = run_bass_kernel_spmd(nc, in_maps, list(range(N_CORES)))

    y_full = np.empty((N, 1), np.float32)
    x_full = np.empty((N, 1), np.float32)
    for c in range(N_CORES):
        sl = slice(c * n_loc, (c + 1) * n_loc)
        y_full[sl, 0] = res.results[c]["yout"].T.reshape(n_loc)
        x_full[sl, 0] = res.results[c]["xout"].T.reshape(n_loc)

    return (y_full, x_full)
